# revision 39
# baseline (speedup 1.0000x reference)
"""Trainium2 Bass kernel for nn_NeuroManifoldBlock (dense transformer block with
FitzHugh-Nagumo-evolved attention scores), SPMD across 8 NeuronCores.

Sharding: cores 0-3 -> batch 0, cores 4-7 -> batch 1. Within a batch group of
4 cores: the sdr projection is feature-sharded and joined by a bf16 on-chip
AllGather; attention is head-sharded (4 heads/core); the residual+MLP are
token-sharded (256 tokens/core), fed by two ReduceScatters (token halves)
that sum the per-head out-projection partials PLUS 0.25x per core (so the
residual stream x rides through the collective for free).

Everything after the AllGather stays feature-major ([feature, token] tiles):
out-projection, residual, LN2, SwiGLU, down-projection, and the final output
(un-transposed on the host). This removes all on-chip transposes.

Key perf choices vs the v0 kernel (424us):
 - ~45 large DMAs instead of 403 small ones (HWDGE serializes at ~625ns/DMA;
   the MLP weights alone were 220 DMAs = 137us of HWDGE).
 - FHN IMEX evaluated as a deg-7 poly (Relu clamp -> Act Square -> 2 custom
   DVE Horner passes -> Act Exp) on merged [128,2048] score tiles.
 - Softmax denominator folded into the PV matmul: V tiles carry an appended
   ones column per head ([128, 4x65] interleaved), so ctx_psum row 64 is the
   denominator (no separate ones-matmuls).
 - Causal masking and ctx normalization run on the (otherwise idle) GPSIMD
   engine.
 - LayerNorms never materialize normalized activations in token-major form:
   LN1 folds into QKV as a rank-1 correction + per-token scale (as v0); LN2
   stats come from ones-matmuls over feature-major x2.
"""

import numpy as np
import ml_dtypes

from concourse import bass, bacc, tile
import concourse.mybir as mybir
from concourse.bass_utils import run_bass_kernel_spmd

# ---------------------------------------------------------------- constants
B, T, SDR, D, H, DH = 2, 1024, 2048, 1024, 16, 64
FFN = 2730
FFN_PAD = 2816          # 22 * 128
NFT = 22                # 128-wide FFN tiles
N_CORES = 8
GROUP = 4               # cores per batch
HPC = 4                 # heads per core
TPC = 256               # tokens per core
DT_, FA, FB, FTAU, FTH = 0.1, 0.7, 0.8, 12.5, 0.5
EPS = 1e-5
CLAMP = 3.35
POLY_DEG = 7

F32 = mybir.dt.float32
BF16 = mybir.dt.bfloat16
_bfd = ml_dtypes.bfloat16


def _bf16(x):
    return np.ascontiguousarray(np.asarray(x, np.float32).astype(_bfd))


def _f32(x):
    return np.ascontiguousarray(np.asarray(x, np.float32))


# ------------------------------------------------------- FHN poly (host fit)
def _fhn_g(s):
    s = np.asarray(s, np.float64)
    v = s.copy()
    w = np.zeros_like(s)
    wd = 1.0 + DT_ * FB / FTAU
    for _ in range(4):
        v = v + DT_ * (v - v ** 3 / 3.0 - w + s)
        w = (w + DT_ * (v + FA) / FTAU) / wd
    return v - FTH


def _fit_poly():
    # poly in u = relu(s_raw + 8*CLAMP), s_raw = unscaled scores (q.k)
    xs = np.linspace(0.0, 2 * 8 * CLAMP, 400001)
    g = _fhn_g(xs / 8.0 - CLAMP)
    c = np.polynomial.chebyshev.Chebyshev.fit(xs, g, POLY_DEG)
    return c.convert(kind=np.polynomial.Polynomial).coef[::-1].astype(np.float64)


POLY = _fit_poly()
_c0, _c1, _c2 = POLY[0], POLY[1], POLY[2]
SQ_SIGN = 1.0 if _c0 > 0 else -1.0
_SW = np.sqrt(abs(_c0))
SQ_SCALE = float(_SW)
SQ_BIAS = float(_SW * (_c1 / (2 * _c0)))
SQ_GAMMA = float(_c2 - _c1 ** 2 / (4 * _c0))
HC = [float(c) for c in POLY[3:]]
assert len(HC) == POLY_DEG - 2


# ------------------------------------------------- custom DVE ops (runtime)
def _register_custom_ops():
    from concourse import dve_ops as DO
    from concourse.dve_spec import Spec, Src0, Src1, C0, C1, C2, lower
    from concourse.dve_uop import DveOpSpec

    defs = {
        "ANT_TT_MULT_ADDC": Spec(
            body=Src0 * Src1 + C0,
            reference=lambda in0, in1, s0, s1, imm2: (
                in0.astype(np.float32) * in1 + s0),
        ),
        "ANT_TT_ADDC_MULT": Spec(
            body=(Src0 + C0) * Src1,
            reference=lambda in0, in1, s0, s1, imm2: (
                (in0.astype(np.float32) + s0) * in1),
        ),
        "ANT_MUL_C_ADD_T": Spec(
            body=Src0 * C0 + Src1,
            reference=lambda in0, in1, s0, s1, imm2: (
                in0.astype(np.float32) * s0 + in1),
        ),
        "ANT_AXPY_BC": Spec(
            body=Src0 * C0 + Src1 + C1,
            reference=lambda in0, in1, s0, s1, imm2: (
                in0.astype(np.float32) * s0 + in1 + s1),
        ),
        "ANT_H3_NEG": Spec(
            body=((C0 - Src0) * Src1 + C1) * Src1 + C2,
            reference=lambda in0, in1, s0, s1, imm2: (
                ((s0 - in0.astype(np.float32)) * in1 + s1) * in1 + imm2),
        ),
        "ANT_H3_POS": Spec(
            body=((C0 + Src0) * Src1 + C1) * Src1 + C2,
            reference=lambda in0, in1, s0, s1, imm2: (
                ((s0 + in0.astype(np.float32)) * in1 + s1) * in1 + imm2),
        ),
        "ANT_H3": Spec(
            body=((Src0 * Src1 + C0) * Src1 + C1) * Src1 + C2,
            reference=lambda in0, in1, s0, s1, imm2: (
                ((in0.astype(np.float32) * in1 + s0) * in1 + s1) * in1 + imm2),
        ),
        "ANT_H2": Spec(
            body=(Src0 * Src1 + C0) * Src1 + C1,
            reference=lambda in0, in1, s0, s1, imm2: (
                (in0.astype(np.float32) * in1 + s0) * in1 + s1),
        ),
    }
    existing = {op.name for op in DO.OPS}
    for name, spec in defs.items():
        if name in existing:
            continue
        row = max(DO._SUB_OPCODE_FOR_NAME.values()) + 1
        assert row < 0x20
        DO._SUB_OPCODE_FOR_NAME[name] = row
        shas = {}
        for ver in ("v3", "v4"):
            try:
                shas[ver] = DveOpSpec(
                    name=name, opcode=row, uops=lower(spec, ver=ver),
                    rd1_en=True).sha(ver)
            except Exception:
                pass
        op = DO.DveOp(name, spec, subdim=False, uops_sha=shas)
        DO.OPS.append(op)
        DO.CUSTOM_DVE_SPECS[name] = spec
    return {op.name: op for op in DO.OPS}


_OPS = _register_custom_ops()


# ----------------------------------------------------------- graph builder
def build_graph(debug=False, single=False):
    nc = bacc.Bacc("TRN2", target_bir_lowering=False, debug=False,
                   num_devices=(1 if single else N_CORES))

    # const APs for float biases used by non-Copy activations
    for val in (float(EPS), float(8.0 * CLAMP), float(SQ_BIAS)):
        if (F32, val) not in nc.const_aps.aps:
            t_ = nc.alloc_sbuf_tensor(
                f"const-f32-{abs(hash(val)) % 10**8}", [128, 1], F32)
            nc.gpsimd.memset(t_.ap(), val)
            nc.const_aps.aps[(F32, val)] = t_.ap()
    nc.all_engine_barrier()

    def din(name, shape, dtype):
        return nc.dram_tensor(name, list(shape), dtype, kind="ExternalInput").ap()

    sdr_pk = din("sdr_pk", (128, 16 * 1024), BF16)
    wsdrmy_pk = din("wsdrmy_pk", (128, 16 * 256), BF16)
    sdrb_my = din("sdrb_my", (128, 2), F32)
    wqk_pk = din("wqk_pk", (128, 8 * 512), BF16)
    qk_bias = din("qk_bias", (128, 4), F32)
    qkcs_bf = din("qkcs_bf", (1, 512), BF16)
    wv_pk = din("wv_pk", (128, 8 * 256), BF16)
    vcs_bf = din("vcs_bf", (1, 256), BF16)
    v_bias_bc = din("v_bias_bc", (128, 256), F32)
    wout_pk = din("wout_pk", (128, 2 * 1024), BF16)
    bout_pk = din("bout_pk", (128, 8), F32)
    masks_pk = din("masks_pk", (128, 4 * 512), BF16)
    identf = din("identf", (128, 128), F32)
    wgu_pk = din("wgu_pk", (128, NFT * 2 * 8 * 128), BF16)
    gu_bias = din("gu_bias", (128, 2 * NFT), F32)
    wd_pk = din("wd_pk", (128, 8 * NFT * 128), BF16)

    out_ap = nc.dram_tensor("out_f", [128, 8 * 256], F32,
                            kind="ExternalOutput").ap()
    dbg = {}
    if debug:
        def dout(name, shape, dtype=F32):
            dbg[name] = nc.dram_tensor(name, list(shape), dtype,
                                       kind="ExternalOutput").ap()
        dout("dbg_q", (2 * 128, T), BF16)
        dout("dbg_k", (2 * 128, T), BF16)
        dout("dbg_v", (8 * 128, 260), BF16)
        dout("dbg_ctx", (2 * 128, T), BF16)
        dout("dbg_x2f", (128, 8 * 256), BF16)
        dout("dbg_h2", (128, 8 * 256), BF16)
        dout("dbg_su", (128, NFT * 256), BF16)
        dout("dbg_sil", (128, NFT * 256), BF16)

    NKK = 16       # 128-row chunks of SDR
    NDT = 8        # 128-feature tiles of D

    from concourse.dve_ops import OPS as _ops_list
    OP = {o.name: o for o in _ops_list}
    H3S = OP["ANT_H3_POS"] if SQ_SIGN > 0 else OP["ANT_H3_NEG"]
    AF = mybir.ActivationFunctionType
    ALU = mybir.AluOpType

    with tile.TileContext(nc) as tc:
        pp = tc.alloc_tile_pool(name="persist", bufs=1)
        dram = tc.alloc_tile_pool(name="dram", bufs=1, space="DRAM")
        sp = tc.alloc_tile_pool(name="sdrp", bufs=1)
        psB = tc.alloc_tile_pool(name="psumB", bufs=1, space="PSUM")

        # big input loads first (SP DMA queue is FIFO; sdr chunks gate phase 1)
        wsdrmy_sb = sp.tile([128, 16 * 256], BF16, name="wsdrmy_sb")
        nc.sync.dma_start(wsdrmy_sb[:], wsdrmy_pk[:])
        sdr_sb = sp.tile([128, 16 * 1024], BF16, name="sdr_sb")
        for j4 in range(4):
            nc.sync.dma_start(sdr_sb[:, j4 * 4096:(j4 + 1) * 4096],
                              sdr_pk[:, j4 * 4096:(j4 + 1) * 4096])
        wqk_sb = sp.tile([128, 8 * 512], BF16, name="wqk_sb")
        wv_sb = sp.tile([128, 8 * 256], BF16, name="wv_sb")
        r_bcast = sp.tile([128, T], F32, name="r_bcast")
        negmu_bf = sp.tile([1, T], BF16, name="negmu_bf")
        r_col = [sp.tile([128, 1], F32, name=f"r_col{i}") for i in range(8)]

        # ---------------- persistent small tiles ----------------
        ones_col = pp.tile([128, 1], BF16, name="ones_col")
        nc.vector.memset(ones_col[:], 1.0)
        ones_row_f = pp.tile([1, 128], F32, name="ones_row_f")
        nc.vector.memset(ones_row_f[:], 1.0)
        ones_row_bf = pp.tile([1, 128], BF16, name="ones_row_bf")
        nc.vector.memset(ones_row_bf[:], 1.0)
        identf_sb = sp.tile([128, 128], F32, name="identf_sb")
        nc.sync.dma_start(identf_sb[:], identf[:])

        sdrbmy_sb = sp.tile([128, 2], F32, name="sdrbmy_sb")
        nc.sync.dma_start(sdrbmy_sb[:], sdrb_my[:])
        qkb_sb = sp.tile([128, 4], F32, name="qkb_sb")
        nc.sync.dma_start(qkb_sb[:], qk_bias[:])
        qkcs_sb = sp.tile([1, 512], BF16, name="qkcs_sb")
        nc.sync.dma_start(qkcs_sb[:], qkcs_bf[:])
        vcs_sb = sp.tile([1, 256], BF16, name="vcs_sb")
        nc.sync.dma_start(vcs_sb[:], vcs_bf[:])
        vbias_sb = sp.tile([128, 256], F32, name="vbias_sb")
        nc.sync.dma_start(vbias_sb[:], v_bias_bc[:])
        bout_sb = pp.tile([128, 8], F32, name="bout_sb")
        nc.sync.dma_start(bout_sb[:], bout_pk[:])
        masks_sb = pp.tile([128, 2048], BF16, name="masks_sb")
        nc.sync.dma_start(masks_sb[:], masks_pk[:])
        gub_sb = pp.tile([128, 2 * NFT], F32, name="gub_sb")
        nc.sync.dma_start(gub_sb[:], gu_bias[:])

        wout_sb = pp.tile([128, 2 * 1024], BF16, name="wout_sb")
        nc.sync.dma_start(wout_sb[:], wout_pk[:])
        x_bf = pp.tile([128, 8 * 1024], BF16, name="x_bf")
        qhp = [pp.tile([128, T], BF16, name=f"qhp{i}") for i in range(2)]
        khp = [pp.tile([128, T], BF16, name=f"khp{i}") for i in range(2)]
        vts = [pp.tile([128, 260], BF16, name=f"vts{i}") for i in range(8)]
        for vt in range(8):
            nc.vector.memset(vts[vt][:], 1.0)
        ctx_hp = [pp.tile([128, T], BF16, name=f"ctx_hp{i}") for i in range(2)]

        # dram staging
        ag_in = dram.tile([256, T], BF16, name="ag_in")
        ag_out = dram.tile([D, T], BF16, name="ag_out")
        b_in = [dram.tile([GROUP * 128, 8, 128], BF16, name=f"b_in{i}")
                for i in range(2)]
        b_out = [dram.tile([128, 8, 128], BF16, name=f"b_out{i}")
                 for i in range(2)]

        # ---------------- phase 1: sdr projection ----------------
        for dt2 in range(2):
            ps = psB.tile([128, 1024], F32, name="mm", tag="mm", bufs=2)
            for h5 in range(2):
                for kk in range(NKK):
                    nc.tensor.matmul(
                        ps[:, h5 * 512:(h5 + 1) * 512],
                        wsdrmy_sb[:, kk * 256 + dt2 * 128:
                                  kk * 256 + (dt2 + 1) * 128],
                        sdr_sb[:, kk * 1024 + h5 * 512:kk * 1024 + (h5 + 1) * 512],
                        start=(kk == 0), stop=(kk == NKK - 1))
            xout = sp.tile([128, 1024], BF16, name="xout", tag="xout", bufs=2)
            nc.scalar.activation(xout[:], ps[:], AF.Identity,
                                 bias=sdrbmy_sb[:, dt2:dt2 + 1])
            nc.sync.dma_start(ag_in[dt2 * 128:(dt2 + 1) * 128, :], xout[:])

        if single:
            for r in range(4):
                nc.sync.dma_start(ag_out[r * 256:(r + 1) * 256, :], ag_in[:])
                for dd in (2 * r, 2 * r + 1):
                    nc.sync.dma_start(x_bf[:, dd * 1024:(dd + 1) * 1024],
                                      ag_out[dd * 128:(dd + 1) * 128, :])
        else:
            nc.gpsimd.collective_compute(
                "AllGather", mybir.AluOpType.bypass,
                ins=[ag_in.opt()], outs=[ag_out.opt()],
                replica_groups=[[0, 1, 2, 3], [4, 5, 6, 7]])
            for dd in range(NDT):
                nc.sync.dma_start(x_bf[:, dd * 1024:(dd + 1) * 1024],
                                  ag_out[dd * 128:(dd + 1) * 128, :])
        nc.sync.dma_start(wqk_sb[:], wqk_pk[:])
        nc.sync.dma_start(wv_sb[:], wv_pk[:])

        # ---------------- LN1 stats from gathered x ----------------
        mu_ps = psB.tile([128, 1024], F32, name="mu_ps", tag="st0")
        sxx_ps = psB.tile([128, 1024], F32, name="sxx_ps", tag="st1")
        for dt_i in range(NDT):
            xsq = sp.tile([128, 1024], BF16, name="xsq", tag="xsq", bufs=3)
            nc.vector.tensor_tensor(
                xsq[:], x_bf[:, dt_i * 1024:(dt_i + 1) * 1024],
                x_bf[:, dt_i * 1024:(dt_i + 1) * 1024], op=ALU.mult)
            for h5 in range(2):
                nc.tensor.matmul(
                    mu_ps[0:1, h5 * 512:(h5 + 1) * 512], ones_col[:],
                    x_bf[:, dt_i * 1024 + h5 * 512:dt_i * 1024 + (h5 + 1) * 512],
                    start=(dt_i == 0), stop=(dt_i == NDT - 1))
                nc.tensor.matmul(
                    sxx_ps[0:1, h5 * 512:(h5 + 1) * 512], ones_col[:],
                    xsq[:, h5 * 512:(h5 + 1) * 512],
                    start=(dt_i == 0), stop=(dt_i == NDT - 1))
        mu_row = sp.tile([1, T], F32, name="mu_row")
        nc.scalar.activation(mu_row[:], mu_ps[0:1, :], AF.Copy, scale=1.0 / D)
        sxx_row = sp.tile([1, T], F32, name="sxx_row")
        nc.scalar.activation(sxx_row[:], sxx_ps[0:1, :], AF.Copy, scale=1.0 / D)

        negmu_row = sp.tile([1, T], F32, name="negmu_row")
        nc.vector.tensor_scalar(negmu_row[:], mu_row[:], -1.0, None,
                                op0=ALU.mult)
        nc.scalar.activation(negmu_bf[:], negmu_row[:], AF.Copy)
        musq = sp.tile([1, T], F32, name="musq", tag="rowtmp", bufs=2)
        nc.vector.tensor_tensor(musq[:], mu_row[:], mu_row[:], op=ALU.mult)
        var_row = sp.tile([1, T], F32, name="var_row", tag="rowtmp", bufs=2)
        nc.vector.tensor_tensor(var_row[:], sxx_row[:], musq[:],
                                op=ALU.subtract)
        lnv = sp.tile([1, T], F32, name="lnv", tag="rowtmp", bufs=2)
        nc.scalar.activation(lnv[:], var_row[:], AF.Ln, bias=EPS)
        r_row = sp.tile([1, T], F32, name="r_row", tag="rowtmp", bufs=2)
        nc.scalar.activation(r_row[:], lnv[:], AF.Exp, scale=-0.5)
        rb_ps = psB.tile([128, 1024], F32, name="rb_ps", tag="mm", bufs=2)
        for h5 in range(2):
            nc.tensor.matmul(rb_ps[:, h5 * 512:(h5 + 1) * 512], ones_row_f[:],
                             r_row[:, h5 * 512:(h5 + 1) * 512])
        nc.scalar.activation(r_bcast[:], rb_ps[:], AF.Copy)
        for j in range(8):
            tp = psB.tile([128, 1024], F32, name="tp", tag="mm", bufs=2)
            nc.tensor.transpose(tp[:, 0:128], r_bcast[:, j * 128:(j + 1) * 128],
                                identf_sb[:])
            nc.vector.tensor_copy(r_col[j][:], tp[:, 0:1])

        # ---------------- phase 2: qkv ----------------
        for fp in range(4):
            ps = psB.tile([128, 1024], F32, name="qk_ps", tag="mm", bufs=2)
            for h5 in range(2):
                for kk in range(NDT):
                    nc.tensor.matmul(
                        ps[:, h5 * 512:(h5 + 1) * 512],
                        wqk_sb[:, kk * 512 + fp * 128:kk * 512 + (fp + 1) * 128],
                        x_bf[:, kk * 1024 + h5 * 512:kk * 1024 + (h5 + 1) * 512],
                        start=(kk == 0), stop=False)
                nc.tensor.matmul(
                    ps[:, h5 * 512:(h5 + 1) * 512],
                    qkcs_sb[:, fp * 128:(fp + 1) * 128],
                    negmu_bf[:, h5 * 512:(h5 + 1) * 512],
                    start=False, stop=True)
            dst = (qhp if fp < 2 else khp)[fp % 2]
            nc.vector._custom_dve(
                OP["ANT_TT_MULT_ADDC"], out=dst[:], in0=ps[:], in1=r_bcast[:],
                s0=qkb_sb[:, fp:fp + 1])

        for vt in range(8):
            ps = psB.tile([128, 1024], F32, name="v_ps", tag="mm", bufs=2)
            for kk in range(NDT):
                nc.tensor.matmul(
                    ps[:, 0:256],
                    x_bf[:, kk * 1024 + vt * 128:kk * 1024 + (vt + 1) * 128],
                    wv_sb[:, kk * 256:(kk + 1) * 256],
                    start=(kk == 0), stop=False)
            nc.tensor.matmul(
                ps[:, 0:256], negmu_bf[:, vt * 128:(vt + 1) * 128], vcs_sb[:],
                start=False, stop=True)
            vdst = vts[vt][:, 0:260].rearrange("p (h c) -> p h c", c=65)
            nc.vector._custom_dve(
                OP["ANT_MUL_C_ADD_T"], out=vdst[:, :, 0:64], in0=ps[:, 0:256],
                in1=vbias_sb[:], s0=r_col[vt][:])

        if debug:
            for i in range(2):
                nc.sync.dma_start(dbg["dbg_q"][i * 128:(i + 1) * 128, :],
                                  qhp[i][:])
                nc.sync.dma_start(dbg["dbg_k"][i * 128:(i + 1) * 128, :],
                                  khp[i][:])
            for vt in range(8):
                nc.sync.dma_start(dbg["dbg_v"][vt * 128:(vt + 1) * 128, :],
                                  vts[vt][:])

        psB.release()
        sp.release()

        # ---------------- phase 3: attention + outproj + RS + LN2 ------
        mlp_pool = tc.alloc_tile_pool(name="mlp", bufs=1)
        mw = tc.alloc_tile_pool(name="mlpw", bufs=1)
        ap_ = tc.alloc_tile_pool(name="attn", bufs=1)
        psO = tc.alloc_tile_pool(name="psumO", bufs=1, space="PSUM")
        apA = tc.alloc_tile_pool(name="attnA", bufs=1)
        psA = tc.alloc_tile_pool(name="psumA", bufs=1, space="PSUM")

        po_sb = mlp_pool.tile([128, 8 * 512], BF16, name="po_sb")
        x2f = mlp_pool.tile([128, 8 * 256], BF16, name="x2f")
        h2 = mlp_pool.tile([128, 8 * 256], BF16, name="h2")

        # MLP weight prefetch: lands during the attention window (DMA idle)
        wd_sb0 = mw.tile([128, 11 * 8 * 128], BF16, name="wd_sb0")
        nc.sync.dma_start(wd_sb0[:], wd_pk[:, 0:11264])
        wd_sb1 = mw.tile([128, 11 * 8 * 128], BF16, name="wd_sb1")
        nc.sync.dma_start(wd_sb1[:], wd_pk[:, 11264:22528])
        wgu_tiles = {}
        def _wgu_load(jft):
            t_ = mw.tile([128, 4096], BF16, name="wgu_t", tag="wgu", bufs=3)
            nc.sync.dma_start(t_[:], wgu_pk[:, jft * 4096:(jft + 1) * 4096])
            wgu_tiles[jft] = t_
        for jft in range(3):
            _wgu_load(jft)

        for qt in range(2):
            # -------- FHN attention for query block qt (512 tokens) -----
            for h in range(HPC):
                hp, hb = h // 2, (h % 2) * 64
                nkt = 4 * (qt + 1)
                ctx_ps = psA.tile([65, 512], F32, name="ctx_ps", tag="ctx",
                                  bufs=1)
                for mac in range(qt + 1):
                    kts = list(range(mac * 4, mac * 4 + 4))
                    sc_ps = psA.tile([128, 2048], F32, name="sc_ps", tag="sc",
                                     bufs=1)
                    for i, kt in enumerate(kts):
                        nc.tensor.matmul(
                            sc_ps[:, i * 512:(i + 1) * 512],
                            khp[hp][hb:hb + 64, kt * 128:(kt + 1) * 128],
                            qhp[hp][hb:hb + 64, qt * 512:(qt + 1) * 512])
                    u_buf = apA.tile([128, 2048], F32, name="u_buf",
                                     tag="u_buf", bufs=3)
                    nc.scalar.activation(u_buf[:], sc_ps[:], AF.Relu,
                                         bias=8.0 * CLAMP)
                    h_buf = apA.tile([128, 2048], F32, name="h_buf",
                                     tag="h_buf", bufs=3)
                    nc.scalar.activation(h_buf[:], u_buf[:], AF.Square,
                                         bias=SQ_BIAS, scale=SQ_SCALE)
                    nc.vector._custom_dve(
                        H3S, out=h_buf[:], in0=h_buf[:], in1=u_buf[:],
                        s0=SQ_GAMMA, s1=HC[0], imm2=HC[1])
                    nc.vector._custom_dve(
                        OP["ANT_H3"], out=h_buf[:], in0=h_buf[:],
                        in1=u_buf[:], s0=HC[2], s1=HC[3], imm2=HC[4])
                    if POLY_DEG == 10:
                        nc.vector._custom_dve(
                            OP["ANT_H3"], out=h_buf[:], in0=h_buf[:],
                            in1=u_buf[:], s0=HC[5], s1=HC[6], imm2=HC[7])
                    p_buf = apA.tile([128, 2048], BF16, name="p_buf",
                                     tag="p_buf", bufs=3)
                    nc.scalar.activation(p_buf[:], h_buf[:], AF.Exp)
                    for i, kt in enumerate(kts):
                        dv = kt * 128 - qt * 512
                        if dv >= 0:
                            nc.gpsimd.tensor_tensor(
                                p_buf[:, i * 512:(i + 1) * 512],
                                p_buf[:, i * 512:(i + 1) * 512],
                                masks_sb[:, (dv // 128) * 512:
                                         (dv // 128 + 1) * 512],
                                op=ALU.mult)
                    for i, kt in enumerate(kts):
                        first = (mac == 0 and i == 0)
                        last = (mac == qt and i == 3)
                        nc.tensor.matmul(
                            ctx_ps[:], vts[kt][:, h * 65:(h + 1) * 65],
                            p_buf[:, i * 512:(i + 1) * 512],
                            start=first, stop=last)
                den_sb = apA.tile([1, 512], F32, name="den_sb", tag="den",
                                  bufs=1)
                nc.scalar.activation(den_sb[:], ctx_ps[64:65, :], AF.Copy)
                rec_sb = apA.tile([1, 512], F32, name="rec_sb", tag="rec",
                                  bufs=1)
                nc.vector.reciprocal_approx_fast(rec_sb[:], den_sb[:])
                rec_bf = apA.tile([1, 512], BF16, name="rec_bf", tag="recb",
                                  bufs=1)
                nc.scalar.activation(rec_bf[:], rec_sb[:], AF.Copy)
                recb_ps = psA.tile([64, 512], F32, name="recb_ps", tag="rb",
                                   bufs=1)
                nc.tensor.matmul(recb_ps[:], ones_row_bf[:, 0:64], rec_bf[:])
                recb_sb = apA.tile([64, 512], BF16, name="recb_sb",
                                   tag="recbs", bufs=2)
                nc.scalar.activation(recb_sb[:], recb_ps[:], AF.Copy)
                nc.vector.tensor_tensor(
                    ctx_hp[hp][hb:hb + 64, qt * 512:(qt + 1) * 512],
                    ctx_ps[0:64, :], recb_sb[:], op=ALU.mult)

            # -------- out-projection for this token half (feature-major) --
            half = qt
            for dd in range(NDT):
                ps_op = psO.tile([128, 512], F32, name="op_ps", tag="op",
                                 bufs=2)
                for hp2 in range(2):
                    nc.tensor.matmul(
                        ps_op[:],
                        wout_sb[:, hp2 * 1024 + dd * 128:
                                hp2 * 1024 + (dd + 1) * 128],
                        ctx_hp[hp2][:, half * 512:(half + 1) * 512],
                        start=(hp2 == 0), stop=(hp2 == 1))
                nc.vector._custom_dve(
                    OP["ANT_AXPY_BC"],
                    out=po_sb[:, dd * 512:(dd + 1) * 512],
                    in0=x_bf[:, dd * 1024 + half * 512:
                             dd * 1024 + (half + 1) * 512],
                    in1=ps_op[:], s0=1.0 / GROUP, s1=bout_sb[:, dd:dd + 1])
            po3 = po_sb[:].rearrange("p (d t) -> p d t", d=8)
            for r in range(GROUP):
                nc.sync.dma_start(
                    b_in[half][r * 128:(r + 1) * 128],
                    po3[:, :, r * 128:(r + 1) * 128])
            if single:
                nc.sync.dma_start(b_out[half][:], b_in[half][0:128])
            else:
                nc.gpsimd.collective_compute(
                    "ReduceScatter", mybir.AluOpType.add,
                    ins=[b_in[half].opt()], outs=[b_out[half].opt()],
                    replica_groups=[[0, 1, 2, 3], [4, 5, 6, 7]])
            x2f3 = x2f[:].rearrange("p (d t) -> p d t", d=8)
            nc.sync.dma_start(x2f3[:, :, half * 128:(half + 1) * 128],
                              b_out[half][:])

            # -------- LN2 for this half (feature-major) ------------------
            st_ps = psO.tile([128, 512], F32, name="st_ps", tag="op", bufs=2)
            for dd in range(NDT):
                xs2 = ap_.tile([128, 128], BF16, name="xs2", tag="xs2", bufs=3)
                sl = x2f[:, dd * 256 + half * 128:dd * 256 + (half + 1) * 128]
                nc.vector.tensor_tensor(xs2[:], sl, sl, op=ALU.mult)
                nc.tensor.matmul(st_ps[0:1, 0:128], ones_col[:], sl,
                                 start=(dd == 0), stop=(dd == NDT - 1))
                nc.tensor.matmul(st_ps[0:1, 128:256], ones_col[:], xs2[:],
                                 start=False, stop=(dd == NDT - 1))
            mu2 = ap_.tile([1, 128], F32, name="mu2", tag="mu2", bufs=2)
            nc.scalar.activation(mu2[:], st_ps[0:1, 0:128], AF.Copy,
                                 scale=1.0 / D)
            sxx2 = ap_.tile([1, 128], F32, name="sxx2", tag="sxx2", bufs=2)
            nc.scalar.activation(sxx2[:], st_ps[0:1, 128:256], AF.Copy,
                                 scale=1.0 / D)
            musq2 = ap_.tile([1, 128], F32, name="musq2", tag="mu2t", bufs=2)
            nc.vector.tensor_tensor(musq2[:], mu2[:], mu2[:], op=ALU.mult)
            var2 = ap_.tile([1, 128], F32, name="var2", tag="mu2t", bufs=2)
            nc.vector.tensor_tensor(var2[:], sxx2[:], musq2[:],
                                    op=ALU.subtract)
            lnv2 = ap_.tile([1, 128], F32, name="lnv2", tag="mu2t", bufs=2)
            nc.scalar.activation(lnv2[:], var2[:], AF.Ln, bias=EPS)
            # rmrow: [r2 | -mu2*r2] packed bf16 row for one broadcast matmul
            rmrow = ap_.tile([1, 256], BF16, name="rmrow", tag="rmrow", bufs=2)
            r2t = ap_.tile([1, 128], F32, name="r2t", tag="mu2t", bufs=2)
            nc.scalar.activation(r2t[:], lnv2[:], AF.Exp, scale=-0.5)
            nc.scalar.activation(rmrow[:, 0:128], r2t[:], AF.Copy)
            nmr2 = ap_.tile([1, 128], F32, name="nmr2", tag="mu2t", bufs=2)
            nc.vector.tensor_tensor(nmr2[:], mu2[:], r2t[:], op=ALU.mult)
            nc.vector.tensor_scalar(nmr2[:], nmr2[:], -1.0, None, op0=ALU.mult)
            nc.scalar.activation(rmrow[:, 128:256], nmr2[:], AF.Copy)
            rb2_ps = psO.tile([128, 512], F32, name="rb2_ps", tag="op", bufs=2)
            nc.tensor.matmul(rb2_ps[:, 0:256], ones_row_bf[:], rmrow[:])
            for dd in range(NDT):
                sl = x2f[:, dd * 256 + half * 128:dd * 256 + (half + 1) * 128]
                dsth = h2[:, dd * 256 + half * 128:dd * 256 + (half + 1) * 128]
                nc.vector._custom_dve(
                    OP["ANT_TT_MULT_ADDC"], out=dsth, in0=sl,
                    in1=rb2_ps[:, 0:128], s0=0.0)
                nc.vector.tensor_tensor(dsth, dsth, rb2_ps[:, 128:256],
                                        op=ALU.add)

        if debug:
            for i in range(2):
                nc.sync.dma_start(dbg["dbg_ctx"][i * 128:(i + 1) * 128, :],
                                  ctx_hp[i][:])
            nc.sync.dma_start(dbg["dbg_x2f"][:], x2f[:])
            nc.sync.dma_start(dbg["dbg_h2"][:], h2[:])

        # ---------------- phase 4: SwiGLU + interleaved down-proj -----
        psA.release()
        apA.release()
        psM = tc.alloc_tile_pool(name="psumM", bufs=1, space="PSUM")
        outp = tc.alloc_tile_pool(name="outp", bufs=1)
        su_sb = outp.tile([128, NFT * 256], BF16, name="su_sb")
        out_sb = outp.tile([128, 8 * 256], F32, name="out_sb")
        wd_sb = [wd_sb0, wd_sb1]
        dn_pairs = [psM.tile([128, 512], F32, name=f"dn_pr{i}")
                    for i in range(4)]
        dn_ps = [dn_pairs[dd // 2][:, (dd % 2) * 256:(dd % 2 + 1) * 256]
                 for dd in range(NDT)]

        for jft in range(11):
            wgu_t = wgu_tiles[jft]
            for f2 in range(2):
                ft = jft * 2 + f2
                gu_ps = psO.tile([128, 512], F32, name="gu_ps", tag="op",
                                 bufs=2)
                for dd in range(NDT):
                    nc.tensor.matmul(
                        gu_ps[:, 0:256],
                        wgu_t[:, (f2 * 2) * 1024 + dd * 128:
                              (f2 * 2) * 1024 + (dd + 1) * 128],
                        h2[:, dd * 256:(dd + 1) * 256],
                        start=(dd == 0), stop=(dd == NDT - 1))
                    nc.tensor.matmul(
                        gu_ps[:, 256:512],
                        wgu_t[:, (f2 * 2 + 1) * 1024 + dd * 128:
                              (f2 * 2 + 1) * 1024 + (dd + 1) * 128],
                        h2[:, dd * 256:(dd + 1) * 256],
                        start=False, stop=(dd == NDT - 1))
                sil = ap_.tile([128, 256], BF16, name="sil", tag="sil", bufs=2)
                nc.scalar.activation(sil[:], gu_ps[:, 0:256], AF.Silu,
                                     bias=gub_sb[:, 2 * ft:2 * ft + 1])
                if debug:
                    nc.sync.dma_start(
                        dbg["dbg_sil"][:, ft * 256:(ft + 1) * 256], sil[:])
                nc.vector._custom_dve(
                    OP["ANT_TT_ADDC_MULT"],
                    out=su_sb[:, ft * 256:(ft + 1) * 256],
                    in0=gu_ps[:, 256:512], in1=sil[:],
                    s0=gub_sb[:, 2 * ft + 1:2 * ft + 2])
                # down-proj accumulation for FFN tile ft rides along
                wds = wd_sb[ft // 11]
                fl = ft % 11
                for dd in range(NDT):
                    nc.tensor.matmul(
                        dn_ps[dd],
                        wds[:, (fl * 8 + dd) * 128:(fl * 8 + dd + 1) * 128],
                        su_sb[:, ft * 256:(ft + 1) * 256],
                        start=(ft == 0 and dd % 2 == 0),
                        stop=(ft == NFT - 1))
            if jft + 3 <= 10:
                _wgu_load(jft + 3)

        if debug:
            nc.sync.dma_start(dbg["dbg_su"][:], su_sb[:])

        for dd in range(NDT):
            nc.vector.tensor_tensor(
                out_sb[:, dd * 256:(dd + 1) * 256], dn_ps[dd],
                x2f[:, dd * 256:(dd + 1) * 256], op=ALU.add)
        nc.sync.dma_start(out_ap[:], out_sb[:])

        outp.release()
        psM.release()
        psO.release()
        ap_.release()
        mw.release()
        mlp_pool.release()
        dram.release()
        pp.release()

    nc.compile()
    return nc


# ------------------------------------------------------------- host prep
def _prep_in_maps(inputs):
    sdr = _f32(inputs["sdr"])
    sdr_w = _f32(inputs["sdr_w"])
    sdr_b = _f32(inputs["sdr_b"])
    w_qkv = _f32(inputs["w_qkv"])
    b_qkv = _f32(inputs["b_qkv"])
    w_out = _f32(inputs["w_out"])
    b_out = _f32(inputs["b_out"])
    ln1_g, ln1_b = _f32(inputs["ln1_g"]), _f32(inputs["ln1_b"])
    ln2_g, ln2_b = _f32(inputs["ln2_g"]), _f32(inputs["ln2_b"])
    w_gate, w_up, w_down = (_f32(inputs["w_gate"]), _f32(inputs["w_up"]),
                            _f32(inputs["w_down"]))

    wqkv_f = w_qkv * ln1_g[:, None]
    bqkv_f = ln1_b @ w_qkv + b_qkv
    wg_f = w_gate * ln2_g[:, None]
    bg_f = ln2_b @ w_gate
    wu_f = w_up * ln2_g[:, None]
    bu_f = ln2_b @ w_up

    wg_p = np.zeros((D, FFN_PAD), np.float32); wg_p[:, :FFN] = wg_f
    wu_p = np.zeros((D, FFN_PAD), np.float32); wu_p[:, :FFN] = wu_f
    wd_p = np.zeros((FFN_PAD, D), np.float32); wd_p[:FFN, :] = w_down
    gb_p = np.zeros((FFN_PAD,), np.float32); gb_p[:FFN] = bg_f
    ub_p = np.zeros((FFN_PAD,), np.float32); ub_p[:FFN] = bu_f

    # wgu_pk[p, ((ft*2+gu)*8+dd)*128 + j] = w{g,u}_p[dd*128+p, ft*128+j]
    wg4 = wg_p.reshape(8, 128, NFT, 128)     # dd, p, ft, j
    wu4 = wu_p.reshape(8, 128, NFT, 128)
    wgu = np.stack([wg4, wu4], axis=0)       # gu, dd, p, ft, j
    wgu_pk = _bf16(wgu.transpose(3, 0, 1, 4, 2).reshape(NFT * 2 * 8 * 128, 128).T)
    # check: index [p, col] with col = ((ft*2+gu)*8+dd)*128+j
    # transpose(3,0,1,4,2) -> ft, gu, dd, j, p ; reshape(22*2*8*128, 128) rows
    # = ((ft*2+gu)*8+dd)*128+j, cols = p ; .T -> [p, col]  OK

    # wd_pk[p, (kk*8+dd)*128 + j] = wd_p[kk*128+p, dd*128+j]
    wd4 = wd_p.reshape(NFT, 128, 8, 128)     # kk, p, dd, j
    wd_pk = _bf16(wd4.transpose(0, 2, 3, 1).reshape(NFT * 8 * 128, 128).T)

    gu_bias = np.zeros((128, 2 * NFT), np.float32)
    for ft in range(NFT):
        gu_bias[:, 2 * ft] = gb_p[ft * 128:(ft + 1) * 128]
        gu_bias[:, 2 * ft + 1] = ub_p[ft * 128:(ft + 1) * 128]

    jj = np.arange(512)[None, :]
    pp_ = np.arange(128)[:, None]
    masks_pk = _bf16(np.concatenate(
        [(jj >= (v * 128 + pp_)).astype(np.float32) for v in range(4)],
        axis=1))
    identf = _f32(np.eye(128, dtype=np.float32))
    wsdr_bf = _bf16(sdr_w)

    def pack16(a, w):
        # [2048, w] -> [128, 16*w]
        return np.ascontiguousarray(
            a.reshape(16, 128, w).transpose(1, 0, 2).reshape(128, 16 * w))

    sdr_pk_by_batch = [
        pack16(_bf16(sdr[b].T), T) for b in range(B)]

    in_maps = []
    for c in range(N_CORES):
        b, g = c // GROUP, c % GROUP
        hs = slice(g * HPC * DH, (g * HPC + HPC) * DH)
        wq_s = wqkv_f[:, 0 * D:1 * D][:, hs]
        wk_s = wqkv_f[:, 1 * D:2 * D][:, hs]
        wv_s = wqkv_f[:, 2 * D:3 * D][:, hs]
        wqk_s = _bf16(np.concatenate([wq_s, wk_s], axis=1))   # [1024, 512]
        wqk_pk = np.ascontiguousarray(
            wqk_s.reshape(8, 128, 512).transpose(1, 0, 2).reshape(128, 4096))
        qk_b = np.concatenate([bqkv_f[0 * D:1 * D][hs],
                               bqkv_f[1 * D:2 * D][hs]])      # [512]
        qk_bias = np.ascontiguousarray(qk_b.reshape(4, 128).T)  # [128, 4]
        qk_cs = wqk_s.astype(np.float32).sum(axis=0)[None, :]
        wv_bf = _bf16(wv_s)                                    # [1024, 256]
        wv_pk = np.ascontiguousarray(
            wv_bf.reshape(8, 128, 256).transpose(1, 0, 2).reshape(128, 2048))
        v_cs = wv_bf.astype(np.float32).sum(axis=0)[None, :]
        v_bias = bqkv_f[2 * D:3 * D][hs]
        # wout_pk[p, hp*1024 + j] = w_out[g*256 + hp*128 + p, j]
        wo = _bf16(w_out[hs, :])                               # [256, 1024]
        wout_pk = np.ascontiguousarray(
            wo.reshape(2, 128, 1024).transpose(1, 0, 2).reshape(128, 2048))
        bout_pk = np.zeros((128, 8), np.float32)
        if g == 0:
            bout_pk[:] = b_out.reshape(8, 128).T
        in_maps.append({
            "sdr_pk": sdr_pk_by_batch[b],
            "wsdrmy_pk": pack16(
                np.ascontiguousarray(wsdr_bf[:, g * 256:(g + 1) * 256]), 256),
            "sdrb_my": np.ascontiguousarray(
                sdr_b[g * 256:(g + 1) * 256].reshape(2, 128).T),
            "wqk_pk": wqk_pk,
            "qk_bias": qk_bias,
            "qkcs_bf": _bf16(qk_cs),
            "wv_pk": wv_pk,
            "vcs_bf": _bf16(v_cs),
            "v_bias_bc": np.ascontiguousarray(
                np.tile(v_bias[None, :], (128, 1)).astype(np.float32)),
            "wout_pk": wout_pk,
            "bout_pk": bout_pk,
            "masks_pk": masks_pk,
            "identf": identf,
            "wgu_pk": wgu_pk,
            "gu_bias": gu_bias,
            "wd_pk": wd_pk,
        })
    return in_maps


_GRAPH_CACHE = {}


def _get_graph(debug=False):
    if debug not in _GRAPH_CACHE:
        _GRAPH_CACHE[debug] = build_graph(debug=debug)
    return _GRAPH_CACHE[debug]


def kernel(**inputs):
    nc = _get_graph(debug=False)
    in_maps = _prep_in_maps(inputs)
    res = run_bass_kernel_spmd(nc, in_maps, core_ids=list(range(N_CORES)))
    out = np.zeros((B, T, D), np.float32)
    for c in range(N_CORES):
        b, g = c // GROUP, c % GROUP
        # out_f[p, dd*256 + half*128 + t'] = out[half*512 + g*128 + t', dd*128 + p]
        arr = res.results[c]["out_f"].reshape(128, 8, 2, 128)
        blk = arr.transpose(2, 3, 1, 0).reshape(2, 128, 1024)
        out[b, g * 128:(g + 1) * 128, :] = blk[0]
        out[b, 512 + g * 128:512 + (g + 1) * 128, :] = blk[1]
    return out


# revision 41
# speedup vs baseline: 1.0266x; 1.0266x over previous
"""Trainium2 Bass kernel for nn_NeuroManifoldBlock (dense transformer block with
FitzHugh-Nagumo-evolved attention scores), SPMD across 8 NeuronCores.

Sharding: cores 0-3 -> batch 0, cores 4-7 -> batch 1. Within a batch group of
4 cores: the sdr projection is feature-sharded and joined by a bf16 on-chip
AllGather; attention is head-sharded (4 heads/core); the residual+MLP are
token-sharded (256 tokens/core), fed by two ReduceScatters (token halves)
that sum the per-head out-projection partials PLUS 0.25x per core (so the
residual stream x rides through the collective for free).

Everything after the AllGather stays feature-major ([feature, token] tiles):
out-projection, residual, LN2, SwiGLU, down-projection, and the final output
(un-transposed on the host). This removes all on-chip transposes.

Key perf choices vs the v0 kernel (424us):
 - ~45 large DMAs instead of 403 small ones (HWDGE serializes at ~625ns/DMA;
   the MLP weights alone were 220 DMAs = 137us of HWDGE).
 - FHN IMEX evaluated as a deg-7 poly (Relu clamp -> Act Square -> 2 custom
   DVE Horner passes -> Act Exp) on merged [128,2048] score tiles.
 - Softmax denominator folded into the PV matmul: V tiles carry an appended
   ones column per head ([128, 4x65] interleaved), so ctx_psum row 64 is the
   denominator (no separate ones-matmuls).
 - Causal masking and ctx normalization run on the (otherwise idle) GPSIMD
   engine.
 - LayerNorms never materialize normalized activations in token-major form:
   LN1 folds into QKV as a rank-1 correction + per-token scale (as v0); LN2
   stats come from ones-matmuls over feature-major x2.
"""

import numpy as np
import ml_dtypes

from concourse import bass, bacc, tile
import concourse.mybir as mybir
from concourse.bass_utils import run_bass_kernel_spmd

# ---------------------------------------------------------------- constants
B, T, SDR, D, H, DH = 2, 1024, 2048, 1024, 16, 64
FFN = 2730
FFN_PAD = 2816          # 22 * 128
NFT = 22                # 128-wide FFN tiles
N_CORES = 8
GROUP = 4               # cores per batch
HPC = 4                 # heads per core
TPC = 256               # tokens per core
DT_, FA, FB, FTAU, FTH = 0.1, 0.7, 0.8, 12.5, 0.5
EPS = 1e-5
CLAMP = 3.35
POLY_DEG = 7

F32 = mybir.dt.float32
BF16 = mybir.dt.bfloat16
_bfd = ml_dtypes.bfloat16


def _bf16(x):
    return np.ascontiguousarray(np.asarray(x, np.float32).astype(_bfd))


def _f32(x):
    return np.ascontiguousarray(np.asarray(x, np.float32))


# ------------------------------------------------------- FHN poly (host fit)
def _fhn_g(s):
    s = np.asarray(s, np.float64)
    v = s.copy()
    w = np.zeros_like(s)
    wd = 1.0 + DT_ * FB / FTAU
    for _ in range(4):
        v = v + DT_ * (v - v ** 3 / 3.0 - w + s)
        w = (w + DT_ * (v + FA) / FTAU) / wd
    return v - FTH


def _fit_poly():
    # poly in u = relu(s_raw + 8*CLAMP), s_raw = unscaled scores (q.k)
    xs = np.linspace(0.0, 2 * 8 * CLAMP, 400001)
    g = _fhn_g(xs / 8.0 - CLAMP)
    c = np.polynomial.chebyshev.Chebyshev.fit(xs, g, POLY_DEG)
    return c.convert(kind=np.polynomial.Polynomial).coef[::-1].astype(np.float64)


POLY = _fit_poly()
_c0, _c1, _c2 = POLY[0], POLY[1], POLY[2]
SQ_SIGN = 1.0 if _c0 > 0 else -1.0
_SW = np.sqrt(abs(_c0))
SQ_SCALE = float(_SW)
SQ_BIAS = float(_SW * (_c1 / (2 * _c0)))
SQ_GAMMA = float(_c2 - _c1 ** 2 / (4 * _c0))
HC = [float(c) for c in POLY[3:]]
assert len(HC) == POLY_DEG - 2


# ------------------------------------------------- custom DVE ops (runtime)
def _register_custom_ops():
    from concourse import dve_ops as DO
    from concourse.dve_spec import Spec, Src0, Src1, C0, C1, C2, lower
    from concourse.dve_uop import DveOpSpec

    defs = {
        "ANT_TT_MULT_ADDC": Spec(
            body=Src0 * Src1 + C0,
            reference=lambda in0, in1, s0, s1, imm2: (
                in0.astype(np.float32) * in1 + s0),
        ),
        "ANT_TT_ADDC_MULT": Spec(
            body=(Src0 + C0) * Src1,
            reference=lambda in0, in1, s0, s1, imm2: (
                (in0.astype(np.float32) + s0) * in1),
        ),
        "ANT_MUL_C_ADD_T": Spec(
            body=Src0 * C0 + Src1,
            reference=lambda in0, in1, s0, s1, imm2: (
                in0.astype(np.float32) * s0 + in1),
        ),
        "ANT_AXPY_BC": Spec(
            body=Src0 * C0 + Src1 + C1,
            reference=lambda in0, in1, s0, s1, imm2: (
                in0.astype(np.float32) * s0 + in1 + s1),
        ),
        "ANT_H3_NEG": Spec(
            body=((C0 - Src0) * Src1 + C1) * Src1 + C2,
            reference=lambda in0, in1, s0, s1, imm2: (
                ((s0 - in0.astype(np.float32)) * in1 + s1) * in1 + imm2),
        ),
        "ANT_H3_POS": Spec(
            body=((C0 + Src0) * Src1 + C1) * Src1 + C2,
            reference=lambda in0, in1, s0, s1, imm2: (
                ((s0 + in0.astype(np.float32)) * in1 + s1) * in1 + imm2),
        ),
        "ANT_H3": Spec(
            body=((Src0 * Src1 + C0) * Src1 + C1) * Src1 + C2,
            reference=lambda in0, in1, s0, s1, imm2: (
                ((in0.astype(np.float32) * in1 + s0) * in1 + s1) * in1 + imm2),
        ),
        "ANT_H2": Spec(
            body=(Src0 * Src1 + C0) * Src1 + C1,
            reference=lambda in0, in1, s0, s1, imm2: (
                (in0.astype(np.float32) * in1 + s0) * in1 + s1),
        ),
    }
    existing = {op.name for op in DO.OPS}
    for name, spec in defs.items():
        if name in existing:
            continue
        row = max(DO._SUB_OPCODE_FOR_NAME.values()) + 1
        assert row < 0x20
        DO._SUB_OPCODE_FOR_NAME[name] = row
        shas = {}
        for ver in ("v3", "v4"):
            try:
                shas[ver] = DveOpSpec(
                    name=name, opcode=row, uops=lower(spec, ver=ver),
                    rd1_en=True).sha(ver)
            except Exception:
                pass
        op = DO.DveOp(name, spec, subdim=False, uops_sha=shas)
        DO.OPS.append(op)
        DO.CUSTOM_DVE_SPECS[name] = spec
    return {op.name: op for op in DO.OPS}


_OPS = _register_custom_ops()


# ----------------------------------------------------------- graph builder
def build_graph(debug=False, single=False):
    nc = bacc.Bacc("TRN2", target_bir_lowering=False, debug=False,
                   num_devices=(1 if single else N_CORES))

    # const APs for float biases used by non-Copy activations
    for val in (float(EPS), float(8.0 * CLAMP), float(SQ_BIAS)):
        if (F32, val) not in nc.const_aps.aps:
            t_ = nc.alloc_sbuf_tensor(
                f"const-f32-{abs(hash(val)) % 10**8}", [128, 1], F32)
            nc.gpsimd.memset(t_.ap(), val)
            nc.const_aps.aps[(F32, val)] = t_.ap()
    nc.all_engine_barrier()

    def din(name, shape, dtype):
        return nc.dram_tensor(name, list(shape), dtype, kind="ExternalInput").ap()

    sdr_pk = din("sdr_pk", (128, 16 * 1024), BF16)
    wsdrmy_pk = din("wsdrmy_pk", (128, 16 * 256), BF16)
    sdrb_my = din("sdrb_my", (128, 2), F32)
    wqk_pk = din("wqk_pk", (128, 8 * 512), BF16)
    qk_bias = din("qk_bias", (128, 4), F32)
    qkcs_bf = din("qkcs_bf", (1, 512), BF16)
    wv_pk = din("wv_pk", (128, 8 * 256), BF16)
    vcs_bf = din("vcs_bf", (1, 256), BF16)
    v_bias_bc = din("v_bias_bc", (128, 256), F32)
    wout_pk = din("wout_pk", (128, 2 * 1024), BF16)
    bout_pk = din("bout_pk", (128, 8), F32)
    masks_pk = din("masks_pk", (128, 4 * 512), BF16)
    identf = din("identf", (128, 128), F32)
    wgu_pk = din("wgu_pk", (128, NFT * 2 * 8 * 128), BF16)
    gu_bias = din("gu_bias", (128, 2 * NFT), F32)
    wd_pk = din("wd_pk", (128, 8 * NFT * 128), BF16)

    out_ap = nc.dram_tensor("out_f", [128, 8 * 256], F32,
                            kind="ExternalOutput").ap()
    dbg = {}
    if debug:
        def dout(name, shape, dtype=F32):
            dbg[name] = nc.dram_tensor(name, list(shape), dtype,
                                       kind="ExternalOutput").ap()
        dout("dbg_q", (2 * 128, T), BF16)
        dout("dbg_k", (2 * 128, T), BF16)
        dout("dbg_v", (8 * 128, 260), BF16)
        dout("dbg_ctx", (2 * 128, T), BF16)
        dout("dbg_x2f", (128, 8 * 256), BF16)
        dout("dbg_h2", (128, 8 * 256), BF16)
        dout("dbg_su", (128, NFT * 256), BF16)
        dout("dbg_sil", (128, NFT * 256), BF16)

    NKK = 16       # 128-row chunks of SDR
    NDT = 8        # 128-feature tiles of D

    from concourse.dve_ops import OPS as _ops_list
    OP = {o.name: o for o in _ops_list}
    H3S = OP["ANT_H3_POS"] if SQ_SIGN > 0 else OP["ANT_H3_NEG"]
    AF = mybir.ActivationFunctionType
    ALU = mybir.AluOpType

    with tile.TileContext(nc) as tc:
        pp = tc.alloc_tile_pool(name="persist", bufs=1)
        dram = tc.alloc_tile_pool(name="dram", bufs=1, space="DRAM")
        sp = tc.alloc_tile_pool(name="sdrp", bufs=1)
        psB = tc.alloc_tile_pool(name="psumB", bufs=1, space="PSUM")

        # big input loads first (SP DMA queue is FIFO; sdr chunks gate phase 1)
        wsdrmy_sb = sp.tile([128, 16 * 256], BF16, name="wsdrmy_sb")
        nc.sync.dma_start(wsdrmy_sb[:], wsdrmy_pk[:])
        sdr_sb = sp.tile([128, 16 * 1024], BF16, name="sdr_sb")
        for j4 in range(4):
            nc.sync.dma_start(sdr_sb[:, j4 * 4096:(j4 + 1) * 4096],
                              sdr_pk[:, j4 * 4096:(j4 + 1) * 4096])
        wqk_sb = sp.tile([128, 8 * 512], BF16, name="wqk_sb")
        wv_sb = sp.tile([128, 8 * 256], BF16, name="wv_sb")
        r_bcast = sp.tile([128, T], F32, name="r_bcast")
        negmu_bf = sp.tile([1, T], BF16, name="negmu_bf")
        r_col = [sp.tile([128, 1], F32, name=f"r_col{i}") for i in range(8)]

        # ---------------- persistent small tiles ----------------
        ones_col = pp.tile([128, 1], BF16, name="ones_col")
        nc.vector.memset(ones_col[:], 1.0)
        ones_row_f = pp.tile([1, 128], F32, name="ones_row_f")
        nc.vector.memset(ones_row_f[:], 1.0)
        ones_row_bf = pp.tile([1, 128], BF16, name="ones_row_bf")
        nc.vector.memset(ones_row_bf[:], 1.0)
        identf_sb = sp.tile([128, 128], F32, name="identf_sb")
        nc.sync.dma_start(identf_sb[:], identf[:])

        sdrbmy_sb = sp.tile([128, 2], F32, name="sdrbmy_sb")
        nc.sync.dma_start(sdrbmy_sb[:], sdrb_my[:])
        qkb_sb = sp.tile([128, 4], F32, name="qkb_sb")
        nc.sync.dma_start(qkb_sb[:], qk_bias[:])
        qkcs_sb = sp.tile([1, 512], BF16, name="qkcs_sb")
        nc.sync.dma_start(qkcs_sb[:], qkcs_bf[:])
        vcs_sb = sp.tile([1, 256], BF16, name="vcs_sb")
        nc.sync.dma_start(vcs_sb[:], vcs_bf[:])
        vbias_sb = sp.tile([128, 256], F32, name="vbias_sb")
        nc.sync.dma_start(vbias_sb[:], v_bias_bc[:])
        bout_sb = pp.tile([128, 8], F32, name="bout_sb")
        nc.sync.dma_start(bout_sb[:], bout_pk[:])
        masks_sb = pp.tile([128, 2048], BF16, name="masks_sb")
        nc.sync.dma_start(masks_sb[:], masks_pk[:])
        gub_sb = pp.tile([128, 2 * NFT], F32, name="gub_sb")
        nc.sync.dma_start(gub_sb[:], gu_bias[:])

        wout_sb = pp.tile([128, 2 * 1024], BF16, name="wout_sb")
        nc.sync.dma_start(wout_sb[:], wout_pk[:])
        x_bf = pp.tile([128, 8 * 1024], BF16, name="x_bf")
        qhp = [pp.tile([128, T], BF16, name=f"qhp{i}") for i in range(2)]
        khp = [pp.tile([128, T], BF16, name=f"khp{i}") for i in range(2)]
        vts = [pp.tile([128, 260], BF16, name=f"vts{i}") for i in range(8)]
        for vt in range(8):
            nc.vector.memset(vts[vt][:], 1.0)
        ctx_hp = [pp.tile([128, T], BF16, name=f"ctx_hp{i}") for i in range(2)]

        # dram staging
        ag_in = dram.tile([256, T], BF16, name="ag_in")
        ag_out = dram.tile([D, T], BF16, name="ag_out")
        b_in = [dram.tile([GROUP * 128, 8, 128], BF16, name=f"b_in{i}")
                for i in range(2)]
        b_out = [dram.tile([128, 8, 128], BF16, name=f"b_out{i}")
                 for i in range(2)]

        # ---------------- phase 1: sdr projection ----------------
        for dt2 in range(2):
            ps = psB.tile([128, 1024], F32, name="mm", tag="mm", bufs=2)
            for h5 in range(2):
                for kk in range(NKK):
                    nc.tensor.matmul(
                        ps[:, h5 * 512:(h5 + 1) * 512],
                        wsdrmy_sb[:, kk * 256 + dt2 * 128:
                                  kk * 256 + (dt2 + 1) * 128],
                        sdr_sb[:, kk * 1024 + h5 * 512:kk * 1024 + (h5 + 1) * 512],
                        start=(kk == 0), stop=(kk == NKK - 1))
            xout = sp.tile([128, 1024], BF16, name="xout", tag="xout", bufs=2)
            nc.scalar.activation(xout[:], ps[:], AF.Identity,
                                 bias=sdrbmy_sb[:, dt2:dt2 + 1])
            nc.sync.dma_start(ag_in[dt2 * 128:(dt2 + 1) * 128, :], xout[:])

        if single:
            for r in range(4):
                nc.sync.dma_start(ag_out[r * 256:(r + 1) * 256, :], ag_in[:])
                for dd in (2 * r, 2 * r + 1):
                    nc.sync.dma_start(x_bf[:, dd * 1024:(dd + 1) * 1024],
                                      ag_out[dd * 128:(dd + 1) * 128, :])
        else:
            nc.gpsimd.collective_compute(
                "AllGather", mybir.AluOpType.bypass,
                ins=[ag_in.opt()], outs=[ag_out.opt()],
                replica_groups=[[0, 1, 2, 3], [4, 5, 6, 7]])
            for dd in range(NDT):
                nc.sync.dma_start(x_bf[:, dd * 1024:(dd + 1) * 1024],
                                  ag_out[dd * 128:(dd + 1) * 128, :])
        nc.sync.dma_start(wqk_sb[:], wqk_pk[:])
        nc.sync.dma_start(wv_sb[:], wv_pk[:])

        # ---------------- LN1 stats from gathered x ----------------
        mu_ps = psB.tile([128, 1024], F32, name="mu_ps", tag="st0")
        sxx_ps = psB.tile([128, 1024], F32, name="sxx_ps", tag="st1")
        for dt_i in range(NDT):
            xsq = sp.tile([128, 1024], BF16, name="xsq", tag="xsq", bufs=3)
            nc.vector.tensor_tensor(
                xsq[:], x_bf[:, dt_i * 1024:(dt_i + 1) * 1024],
                x_bf[:, dt_i * 1024:(dt_i + 1) * 1024], op=ALU.mult)
            for h5 in range(2):
                nc.tensor.matmul(
                    mu_ps[0:1, h5 * 512:(h5 + 1) * 512], ones_col[:],
                    x_bf[:, dt_i * 1024 + h5 * 512:dt_i * 1024 + (h5 + 1) * 512],
                    start=(dt_i == 0), stop=(dt_i == NDT - 1))
                nc.tensor.matmul(
                    sxx_ps[0:1, h5 * 512:(h5 + 1) * 512], ones_col[:],
                    xsq[:, h5 * 512:(h5 + 1) * 512],
                    start=(dt_i == 0), stop=(dt_i == NDT - 1))
        mu_row = sp.tile([1, T], F32, name="mu_row")
        nc.scalar.activation(mu_row[:], mu_ps[0:1, :], AF.Copy, scale=1.0 / D)
        sxx_row = sp.tile([1, T], F32, name="sxx_row")
        nc.scalar.activation(sxx_row[:], sxx_ps[0:1, :], AF.Copy, scale=1.0 / D)

        negmu_row = sp.tile([1, T], F32, name="negmu_row")
        nc.vector.tensor_scalar(negmu_row[:], mu_row[:], -1.0, None,
                                op0=ALU.mult)
        nc.scalar.activation(negmu_bf[:], negmu_row[:], AF.Copy)
        musq = sp.tile([1, T], F32, name="musq", tag="rowtmp", bufs=2)
        nc.vector.tensor_tensor(musq[:], mu_row[:], mu_row[:], op=ALU.mult)
        var_row = sp.tile([1, T], F32, name="var_row", tag="rowtmp", bufs=2)
        nc.vector.tensor_tensor(var_row[:], sxx_row[:], musq[:],
                                op=ALU.subtract)
        lnv = sp.tile([1, T], F32, name="lnv", tag="rowtmp", bufs=2)
        nc.scalar.activation(lnv[:], var_row[:], AF.Ln, bias=EPS)
        r_row = sp.tile([1, T], F32, name="r_row", tag="rowtmp", bufs=2)
        nc.scalar.activation(r_row[:], lnv[:], AF.Exp, scale=-0.5)
        rb_ps = psB.tile([128, 1024], F32, name="rb_ps", tag="mm", bufs=2)
        for h5 in range(2):
            nc.tensor.matmul(rb_ps[:, h5 * 512:(h5 + 1) * 512], ones_row_f[:],
                             r_row[:, h5 * 512:(h5 + 1) * 512])
        nc.scalar.activation(r_bcast[:], rb_ps[:], AF.Copy)
        for j in range(8):
            tp = psB.tile([128, 1024], F32, name="tp", tag="mm", bufs=2)
            nc.tensor.transpose(tp[:, 0:128], r_bcast[:, j * 128:(j + 1) * 128],
                                identf_sb[:])
            nc.vector.tensor_copy(r_col[j][:], tp[:, 0:1])

        # ---------------- phase 2: qkv ----------------
        for fp in range(4):
            ps = psB.tile([128, 1024], F32, name="qk_ps", tag="mm", bufs=2)
            for h5 in range(2):
                for kk in range(NDT):
                    nc.tensor.matmul(
                        ps[:, h5 * 512:(h5 + 1) * 512],
                        wqk_sb[:, kk * 512 + fp * 128:kk * 512 + (fp + 1) * 128],
                        x_bf[:, kk * 1024 + h5 * 512:kk * 1024 + (h5 + 1) * 512],
                        start=(kk == 0), stop=False)
                nc.tensor.matmul(
                    ps[:, h5 * 512:(h5 + 1) * 512],
                    qkcs_sb[:, fp * 128:(fp + 1) * 128],
                    negmu_bf[:, h5 * 512:(h5 + 1) * 512],
                    start=False, stop=True)
            dst = (qhp if fp < 2 else khp)[fp % 2]
            nc.vector._custom_dve(
                OP["ANT_TT_MULT_ADDC"], out=dst[:], in0=ps[:], in1=r_bcast[:],
                s0=qkb_sb[:, fp:fp + 1])

        for vt in range(8):
            ps = psB.tile([128, 1024], F32, name="v_ps", tag="mm", bufs=2)
            for kk in range(NDT):
                nc.tensor.matmul(
                    ps[:, 0:256],
                    x_bf[:, kk * 1024 + vt * 128:kk * 1024 + (vt + 1) * 128],
                    wv_sb[:, kk * 256:(kk + 1) * 256],
                    start=(kk == 0), stop=False)
            nc.tensor.matmul(
                ps[:, 0:256], negmu_bf[:, vt * 128:(vt + 1) * 128], vcs_sb[:],
                start=False, stop=True)
            vdst = vts[vt][:, 0:260].rearrange("p (h c) -> p h c", c=65)
            nc.vector._custom_dve(
                OP["ANT_MUL_C_ADD_T"], out=vdst[:, :, 0:64], in0=ps[:, 0:256],
                in1=vbias_sb[:], s0=r_col[vt][:])

        if debug:
            for i in range(2):
                nc.sync.dma_start(dbg["dbg_q"][i * 128:(i + 1) * 128, :],
                                  qhp[i][:])
                nc.sync.dma_start(dbg["dbg_k"][i * 128:(i + 1) * 128, :],
                                  khp[i][:])
            for vt in range(8):
                nc.sync.dma_start(dbg["dbg_v"][vt * 128:(vt + 1) * 128, :],
                                  vts[vt][:])

        psB.release()
        sp.release()

        # ---------------- phase 3: attention + outproj + RS + LN2 ------
        mlp_pool = tc.alloc_tile_pool(name="mlp", bufs=1)
        mw = tc.alloc_tile_pool(name="mlpw", bufs=1)
        ap_ = tc.alloc_tile_pool(name="attn", bufs=1)
        psO = tc.alloc_tile_pool(name="psumO", bufs=1, space="PSUM")
        apA = tc.alloc_tile_pool(name="attnA", bufs=1)
        psA = tc.alloc_tile_pool(name="psumA", bufs=1, space="PSUM")

        po_sb = mlp_pool.tile([128, 8 * 512], BF16, name="po_sb")
        x2f = mlp_pool.tile([128, 8 * 256], BF16, name="x2f")
        h2 = mlp_pool.tile([128, 8 * 256], BF16, name="h2")
        su_sb = mlp_pool.tile([128, NFT * 256], BF16, name="su_sb")

        # MLP weight prefetch: lands during the attention window (DMA idle)
        wd_sb0 = mw.tile([128, 11 * 8 * 128], BF16, name="wd_sb0")
        nc.sync.dma_start(wd_sb0[:], wd_pk[:, 0:11264])
        wd_sb1 = mw.tile([128, 11 * 8 * 128], BF16, name="wd_sb1")
        nc.sync.dma_start(wd_sb1[:], wd_pk[:, 11264:22528])
        wgu_tiles = {}
        def _wgu_load(jft):
            t_ = mw.tile([128, 4096], BF16, name="wgu_t", tag="wgu", bufs=3)
            nc.sync.dma_start(t_[:], wgu_pk[:, jft * 4096:(jft + 1) * 4096])
            wgu_tiles[jft] = t_
        for jft in range(3):
            _wgu_load(jft)

        for qt in range(2):
            # -------- FHN attention for query block qt (512 tokens) -----
            for h in range(HPC):
                hp, hb = h // 2, (h % 2) * 64
                nkt = 4 * (qt + 1)
                ctx_ps = psA.tile([65, 512], F32, name="ctx_ps", tag="ctx",
                                  bufs=1)
                for mac in range(qt + 1):
                    kts = list(range(mac * 4, mac * 4 + 4))
                    sc_ps = psA.tile([128, 2048], F32, name="sc_ps", tag="sc",
                                     bufs=1)
                    for i, kt in enumerate(kts):
                        nc.tensor.matmul(
                            sc_ps[:, i * 512:(i + 1) * 512],
                            khp[hp][hb:hb + 64, kt * 128:(kt + 1) * 128],
                            qhp[hp][hb:hb + 64, qt * 512:(qt + 1) * 512])
                    u_buf = apA.tile([128, 2048], F32, name="u_buf",
                                     tag="u_buf", bufs=3)
                    nc.scalar.activation(u_buf[:], sc_ps[:], AF.Relu,
                                         bias=8.0 * CLAMP)
                    h_buf = apA.tile([128, 2048], F32, name="h_buf",
                                     tag="h_buf", bufs=2)
                    nc.scalar.activation(h_buf[:], u_buf[:], AF.Square,
                                         bias=SQ_BIAS, scale=SQ_SCALE)
                    nc.vector._custom_dve(
                        H3S, out=h_buf[:], in0=h_buf[:], in1=u_buf[:],
                        s0=SQ_GAMMA, s1=HC[0], imm2=HC[1])
                    nc.vector._custom_dve(
                        OP["ANT_H3"], out=h_buf[:], in0=h_buf[:],
                        in1=u_buf[:], s0=HC[2], s1=HC[3], imm2=HC[4])
                    if POLY_DEG == 10:
                        nc.vector._custom_dve(
                            OP["ANT_H3"], out=h_buf[:], in0=h_buf[:],
                            in1=u_buf[:], s0=HC[5], s1=HC[6], imm2=HC[7])
                    p_buf = apA.tile([128, 2048], BF16, name="p_buf",
                                     tag="p_buf", bufs=3)
                    nc.scalar.activation(p_buf[:], h_buf[:], AF.Exp)
                    for i, kt in enumerate(kts):
                        dv = kt * 128 - qt * 512
                        if dv >= 0:
                            nc.gpsimd.tensor_tensor(
                                p_buf[:, i * 512:(i + 1) * 512],
                                p_buf[:, i * 512:(i + 1) * 512],
                                masks_sb[:, (dv // 128) * 512:
                                         (dv // 128 + 1) * 512],
                                op=ALU.mult)
                    for i, kt in enumerate(kts):
                        first = (mac == 0 and i == 0)
                        last = (mac == qt and i == 3)
                        nc.tensor.matmul(
                            ctx_ps[:], vts[kt][:, h * 65:(h + 1) * 65],
                            p_buf[:, i * 512:(i + 1) * 512],
                            start=first, stop=last)
                den_sb = apA.tile([1, 512], F32, name="den_sb", tag="den",
                                  bufs=1)
                nc.scalar.activation(den_sb[:], ctx_ps[64:65, :], AF.Copy)
                rec_sb = apA.tile([1, 512], F32, name="rec_sb", tag="rec",
                                  bufs=1)
                nc.vector.reciprocal_approx_fast(rec_sb[:], den_sb[:])
                rec_bf = apA.tile([1, 512], BF16, name="rec_bf", tag="recb",
                                  bufs=1)
                nc.scalar.activation(rec_bf[:], rec_sb[:], AF.Copy)
                recb_ps = psA.tile([64, 512], F32, name="recb_ps", tag="rb",
                                   bufs=1)
                nc.tensor.matmul(recb_ps[:], ones_row_bf[:, 0:64], rec_bf[:])
                recb_sb = apA.tile([64, 512], BF16, name="recb_sb",
                                   tag="recbs", bufs=2)
                nc.scalar.activation(recb_sb[:], recb_ps[:], AF.Copy)
                nc.vector.tensor_tensor(
                    ctx_hp[hp][hb:hb + 64, qt * 512:(qt + 1) * 512],
                    ctx_ps[0:64, :], recb_sb[:], op=ALU.mult)

            # -------- out-projection for this token half (feature-major) --
            half = qt
            for dd in range(NDT):
                ps_op = psO.tile([128, 512], F32, name="op_ps", tag="op",
                                 bufs=2)
                for hp2 in range(2):
                    nc.tensor.matmul(
                        ps_op[:],
                        wout_sb[:, hp2 * 1024 + dd * 128:
                                hp2 * 1024 + (dd + 1) * 128],
                        ctx_hp[hp2][:, half * 512:(half + 1) * 512],
                        start=(hp2 == 0), stop=(hp2 == 1))
                nc.vector._custom_dve(
                    OP["ANT_AXPY_BC"],
                    out=po_sb[:, dd * 512:(dd + 1) * 512],
                    in0=x_bf[:, dd * 1024 + half * 512:
                             dd * 1024 + (half + 1) * 512],
                    in1=ps_op[:], s0=1.0 / GROUP, s1=bout_sb[:, dd:dd + 1])
            po3 = po_sb[:].rearrange("p (d t) -> p d t", d=8)
            for r in range(GROUP):
                nc.sync.dma_start(
                    b_in[half][r * 128:(r + 1) * 128],
                    po3[:, :, r * 128:(r + 1) * 128])
            if single:
                nc.sync.dma_start(b_out[half][:], b_in[half][0:128])
            else:
                nc.gpsimd.collective_compute(
                    "ReduceScatter", mybir.AluOpType.add,
                    ins=[b_in[half].opt()], outs=[b_out[half].opt()],
                    replica_groups=[[0, 1, 2, 3], [4, 5, 6, 7]])
            x2f3 = x2f[:].rearrange("p (d t) -> p d t", d=8)
            nc.sync.dma_start(x2f3[:, :, half * 128:(half + 1) * 128],
                              b_out[half][:])

            # -------- LN2 for this half (feature-major) ------------------
            st_ps = psO.tile([128, 512], F32, name="st_ps", tag="op", bufs=2)
            for dd in range(NDT):
                xs2 = ap_.tile([128, 128], BF16, name="xs2", tag="xs2", bufs=3)
                sl = x2f[:, dd * 256 + half * 128:dd * 256 + (half + 1) * 128]
                nc.vector.tensor_tensor(xs2[:], sl, sl, op=ALU.mult)
                nc.tensor.matmul(st_ps[0:1, 0:128], ones_col[:], sl,
                                 start=(dd == 0), stop=(dd == NDT - 1))
                nc.tensor.matmul(st_ps[0:1, 128:256], ones_col[:], xs2[:],
                                 start=False, stop=(dd == NDT - 1))
            mu2 = ap_.tile([1, 128], F32, name="mu2", tag="mu2", bufs=2)
            nc.scalar.activation(mu2[:], st_ps[0:1, 0:128], AF.Copy,
                                 scale=1.0 / D)
            sxx2 = ap_.tile([1, 128], F32, name="sxx2", tag="sxx2", bufs=2)
            nc.scalar.activation(sxx2[:], st_ps[0:1, 128:256], AF.Copy,
                                 scale=1.0 / D)
            musq2 = ap_.tile([1, 128], F32, name="musq2", tag="mu2t", bufs=2)
            nc.vector.tensor_tensor(musq2[:], mu2[:], mu2[:], op=ALU.mult)
            var2 = ap_.tile([1, 128], F32, name="var2", tag="mu2t", bufs=2)
            nc.vector.tensor_tensor(var2[:], sxx2[:], musq2[:],
                                    op=ALU.subtract)
            lnv2 = ap_.tile([1, 128], F32, name="lnv2", tag="mu2t", bufs=2)
            nc.scalar.activation(lnv2[:], var2[:], AF.Ln, bias=EPS)
            # rmrow: [r2 | -mu2*r2] packed bf16 row for one broadcast matmul
            rmrow = ap_.tile([1, 256], BF16, name="rmrow", tag="rmrow", bufs=2)
            r2t = ap_.tile([1, 128], F32, name="r2t", tag="mu2t", bufs=2)
            nc.scalar.activation(r2t[:], lnv2[:], AF.Exp, scale=-0.5)
            nc.scalar.activation(rmrow[:, 0:128], r2t[:], AF.Copy)
            nmr2 = ap_.tile([1, 128], F32, name="nmr2", tag="mu2t", bufs=2)
            nc.vector.tensor_tensor(nmr2[:], mu2[:], r2t[:], op=ALU.mult)
            nc.vector.tensor_scalar(nmr2[:], nmr2[:], -1.0, None, op0=ALU.mult)
            nc.scalar.activation(rmrow[:, 128:256], nmr2[:], AF.Copy)
            rb2_ps = psO.tile([128, 512], F32, name="rb2_ps", tag="op", bufs=2)
            nc.tensor.matmul(rb2_ps[:, 0:256], ones_row_bf[:], rmrow[:])
            for dd in range(NDT):
                sl = x2f[:, dd * 256 + half * 128:dd * 256 + (half + 1) * 128]
                dsth = h2[:, dd * 256 + half * 128:dd * 256 + (half + 1) * 128]
                nc.vector._custom_dve(
                    OP["ANT_TT_MULT_ADDC"], out=dsth, in0=sl,
                    in1=rb2_ps[:, 0:128], s0=0.0)
                nc.vector.tensor_tensor(dsth, dsth, rb2_ps[:, 128:256],
                                        op=ALU.add)

        if debug:
            for i in range(2):
                nc.sync.dma_start(dbg["dbg_ctx"][i * 128:(i + 1) * 128, :],
                                  ctx_hp[i][:])
            nc.sync.dma_start(dbg["dbg_x2f"][:], x2f[:])
            nc.sync.dma_start(dbg["dbg_h2"][:], h2[:])

        # ---------------- phase 4: SwiGLU + interleaved down-proj -----
        psA.release()
        apA.release()
        psM = tc.alloc_tile_pool(name="psumM", bufs=1, space="PSUM")
        outp = tc.alloc_tile_pool(name="outp", bufs=1)
        out_sb = outp.tile([128, 8 * 256], F32, name="out_sb")
        wd_sb = [wd_sb0, wd_sb1]
        dn_pairs = [psM.tile([128, 512], F32, name=f"dn_pr{i}")
                    for i in range(4)]
        dn_ps = [dn_pairs[dd // 2][:, (dd % 2) * 256:(dd % 2 + 1) * 256]
                 for dd in range(NDT)]

        for jft in range(11):
            wgu_t = wgu_tiles[jft]
            for f2 in range(2):
                ft = jft * 2 + f2
                gu_ps = psO.tile([128, 512], F32, name="gu_ps", tag="op",
                                 bufs=2)
                for dd in range(NDT):
                    nc.tensor.matmul(
                        gu_ps[:, 0:256],
                        wgu_t[:, (f2 * 2) * 1024 + dd * 128:
                              (f2 * 2) * 1024 + (dd + 1) * 128],
                        h2[:, dd * 256:(dd + 1) * 256],
                        start=(dd == 0), stop=(dd == NDT - 1))
                    nc.tensor.matmul(
                        gu_ps[:, 256:512],
                        wgu_t[:, (f2 * 2 + 1) * 1024 + dd * 128:
                              (f2 * 2 + 1) * 1024 + (dd + 1) * 128],
                        h2[:, dd * 256:(dd + 1) * 256],
                        start=False, stop=(dd == NDT - 1))
                sil = ap_.tile([128, 256], BF16, name="sil", tag="sil", bufs=2)
                nc.scalar.activation(sil[:], gu_ps[:, 0:256], AF.Silu,
                                     bias=gub_sb[:, 2 * ft:2 * ft + 1])
                if debug:
                    nc.sync.dma_start(
                        dbg["dbg_sil"][:, ft * 256:(ft + 1) * 256], sil[:])
                nc.vector._custom_dve(
                    OP["ANT_TT_ADDC_MULT"],
                    out=su_sb[:, ft * 256:(ft + 1) * 256],
                    in0=gu_ps[:, 256:512], in1=sil[:],
                    s0=gub_sb[:, 2 * ft + 1:2 * ft + 2])
                # down-proj accumulation for FFN tile ft rides along
                wds = wd_sb[ft // 11]
                fl = ft % 11
                for dd in range(NDT):
                    nc.tensor.matmul(
                        dn_ps[dd],
                        wds[:, (fl * 8 + dd) * 128:(fl * 8 + dd + 1) * 128],
                        su_sb[:, ft * 256:(ft + 1) * 256],
                        start=(ft == 0 and dd % 2 == 0),
                        stop=(ft == NFT - 1))
            if jft + 3 <= 10:
                _wgu_load(jft + 3)

        if debug:
            nc.sync.dma_start(dbg["dbg_su"][:], su_sb[:])

        for dd in range(NDT):
            nc.vector.tensor_tensor(
                out_sb[:, dd * 256:(dd + 1) * 256], dn_ps[dd],
                x2f[:, dd * 256:(dd + 1) * 256], op=ALU.add)
            if dd == 3:
                nc.sync.dma_start(out_ap[:, 0:1024], out_sb[:, 0:1024])
        nc.sync.dma_start(out_ap[:, 1024:2048], out_sb[:, 1024:2048])

        outp.release()
        psM.release()
        psO.release()
        ap_.release()
        mw.release()
        mlp_pool.release()
        dram.release()
        pp.release()

    nc.compile()
    return nc


# ------------------------------------------------------------- host prep
def _prep_in_maps(inputs):
    sdr = _f32(inputs["sdr"])
    sdr_w = _f32(inputs["sdr_w"])
    sdr_b = _f32(inputs["sdr_b"])
    w_qkv = _f32(inputs["w_qkv"])
    b_qkv = _f32(inputs["b_qkv"])
    w_out = _f32(inputs["w_out"])
    b_out = _f32(inputs["b_out"])
    ln1_g, ln1_b = _f32(inputs["ln1_g"]), _f32(inputs["ln1_b"])
    ln2_g, ln2_b = _f32(inputs["ln2_g"]), _f32(inputs["ln2_b"])
    w_gate, w_up, w_down = (_f32(inputs["w_gate"]), _f32(inputs["w_up"]),
                            _f32(inputs["w_down"]))

    wqkv_f = w_qkv * ln1_g[:, None]
    bqkv_f = ln1_b @ w_qkv + b_qkv
    wg_f = w_gate * ln2_g[:, None]
    bg_f = ln2_b @ w_gate
    wu_f = w_up * ln2_g[:, None]
    bu_f = ln2_b @ w_up

    wg_p = np.zeros((D, FFN_PAD), np.float32); wg_p[:, :FFN] = wg_f
    wu_p = np.zeros((D, FFN_PAD), np.float32); wu_p[:, :FFN] = wu_f
    wd_p = np.zeros((FFN_PAD, D), np.float32); wd_p[:FFN, :] = w_down
    gb_p = np.zeros((FFN_PAD,), np.float32); gb_p[:FFN] = bg_f
    ub_p = np.zeros((FFN_PAD,), np.float32); ub_p[:FFN] = bu_f

    # wgu_pk[p, ((ft*2+gu)*8+dd)*128 + j] = w{g,u}_p[dd*128+p, ft*128+j]
    wg4 = wg_p.reshape(8, 128, NFT, 128)     # dd, p, ft, j
    wu4 = wu_p.reshape(8, 128, NFT, 128)
    wgu = np.stack([wg4, wu4], axis=0)       # gu, dd, p, ft, j
    wgu_pk = _bf16(wgu.transpose(3, 0, 1, 4, 2).reshape(NFT * 2 * 8 * 128, 128).T)
    # check: index [p, col] with col = ((ft*2+gu)*8+dd)*128+j
    # transpose(3,0,1,4,2) -> ft, gu, dd, j, p ; reshape(22*2*8*128, 128) rows
    # = ((ft*2+gu)*8+dd)*128+j, cols = p ; .T -> [p, col]  OK

    # wd_pk[p, (kk*8+dd)*128 + j] = wd_p[kk*128+p, dd*128+j]
    wd4 = wd_p.reshape(NFT, 128, 8, 128)     # kk, p, dd, j
    wd_pk = _bf16(wd4.transpose(0, 2, 3, 1).reshape(NFT * 8 * 128, 128).T)

    gu_bias = np.zeros((128, 2 * NFT), np.float32)
    for ft in range(NFT):
        gu_bias[:, 2 * ft] = gb_p[ft * 128:(ft + 1) * 128]
        gu_bias[:, 2 * ft + 1] = ub_p[ft * 128:(ft + 1) * 128]

    jj = np.arange(512)[None, :]
    pp_ = np.arange(128)[:, None]
    masks_pk = _bf16(np.concatenate(
        [(jj >= (v * 128 + pp_)).astype(np.float32) for v in range(4)],
        axis=1))
    identf = _f32(np.eye(128, dtype=np.float32))
    wsdr_bf = _bf16(sdr_w)

    def pack16(a, w):
        # [2048, w] -> [128, 16*w]
        return np.ascontiguousarray(
            a.reshape(16, 128, w).transpose(1, 0, 2).reshape(128, 16 * w))

    sdr_pk_by_batch = [
        pack16(_bf16(sdr[b].T), T) for b in range(B)]

    in_maps = []
    for c in range(N_CORES):
        b, g = c // GROUP, c % GROUP
        hs = slice(g * HPC * DH, (g * HPC + HPC) * DH)
        wq_s = wqkv_f[:, 0 * D:1 * D][:, hs]
        wk_s = wqkv_f[:, 1 * D:2 * D][:, hs]
        wv_s = wqkv_f[:, 2 * D:3 * D][:, hs]
        wqk_s = _bf16(np.concatenate([wq_s, wk_s], axis=1))   # [1024, 512]
        wqk_pk = np.ascontiguousarray(
            wqk_s.reshape(8, 128, 512).transpose(1, 0, 2).reshape(128, 4096))
        qk_b = np.concatenate([bqkv_f[0 * D:1 * D][hs],
                               bqkv_f[1 * D:2 * D][hs]])      # [512]
        qk_bias = np.ascontiguousarray(qk_b.reshape(4, 128).T)  # [128, 4]
        qk_cs = wqk_s.astype(np.float32).sum(axis=0)[None, :]
        wv_bf = _bf16(wv_s)                                    # [1024, 256]
        wv_pk = np.ascontiguousarray(
            wv_bf.reshape(8, 128, 256).transpose(1, 0, 2).reshape(128, 2048))
        v_cs = wv_bf.astype(np.float32).sum(axis=0)[None, :]
        v_bias = bqkv_f[2 * D:3 * D][hs]
        # wout_pk[p, hp*1024 + j] = w_out[g*256 + hp*128 + p, j]
        wo = _bf16(w_out[hs, :])                               # [256, 1024]
        wout_pk = np.ascontiguousarray(
            wo.reshape(2, 128, 1024).transpose(1, 0, 2).reshape(128, 2048))
        bout_pk = np.zeros((128, 8), np.float32)
        if g == 0:
            bout_pk[:] = b_out.reshape(8, 128).T
        in_maps.append({
            "sdr_pk": sdr_pk_by_batch[b],
            "wsdrmy_pk": pack16(
                np.ascontiguousarray(wsdr_bf[:, g * 256:(g + 1) * 256]), 256),
            "sdrb_my": np.ascontiguousarray(
                sdr_b[g * 256:(g + 1) * 256].reshape(2, 128).T),
            "wqk_pk": wqk_pk,
            "qk_bias": qk_bias,
            "qkcs_bf": _bf16(qk_cs),
            "wv_pk": wv_pk,
            "vcs_bf": _bf16(v_cs),
            "v_bias_bc": np.ascontiguousarray(
                np.tile(v_bias[None, :], (128, 1)).astype(np.float32)),
            "wout_pk": wout_pk,
            "bout_pk": bout_pk,
            "masks_pk": masks_pk,
            "identf": identf,
            "wgu_pk": wgu_pk,
            "gu_bias": gu_bias,
            "wd_pk": wd_pk,
        })
    return in_maps


_GRAPH_CACHE = {}


def _get_graph(debug=False):
    if debug not in _GRAPH_CACHE:
        _GRAPH_CACHE[debug] = build_graph(debug=debug)
    return _GRAPH_CACHE[debug]


def kernel(**inputs):
    nc = _get_graph(debug=False)
    in_maps = _prep_in_maps(inputs)
    res = run_bass_kernel_spmd(nc, in_maps, core_ids=list(range(N_CORES)))
    out = np.zeros((B, T, D), np.float32)
    for c in range(N_CORES):
        b, g = c // GROUP, c % GROUP
        # out_f[p, dd*256 + half*128 + t'] = out[half*512 + g*128 + t', dd*128 + p]
        arr = res.results[c]["out_f"].reshape(128, 8, 2, 128)
        blk = arr.transpose(2, 3, 1, 0).reshape(2, 128, 1024)
        out[b, g * 128:(g + 1) * 128, :] = blk[0]
        out[b, 512 + g * 128:512 + (g + 1) * 128, :] = blk[1]
    return out
